# revision 7
# baseline (speedup 1.0000x reference)
"""Trainium2 Bass kernel for nn_DGCRM_88227218194820.

The reference module's dynamic-adjacency branch (gconv_hyper / nodevec /
adp) is dead code w.r.t. the returned hidden state: due to the faithful
source bug, gconv_rnn(inp, i) == concat([inp, a*inp, a*inp], -1) @ rnn_W[i]
+ rnn_b[i] uses no adjacency, and the normalized adjacencies are deleted.
The output therefore reduces to a per-row GRU gate:

    combined = concat(x, h)                      # [.., 66]
    z  = sigmoid(combined @ Wz + bz)
    r  = sigmoid(combined @ Wr + br)
    hc = tanh(concat(x, r*h) @ Wc + bc)
    out = z*h + (1-z)*hc

with Wg folded from rnn_W: Wg = W[:66] + a*(W[66:132] + W[132:198]),
summed over the two gconv_rnn calls per gate.

Layout (per core, data-parallel over batch: 2 of 16 batches per core,
R = 2048 rows): everything lives transposed (channels on partitions) and
"group-stacked" -- rows 0:1024 (group A) on partitions 0:64, rows
1024:2048 (group B) on partitions 64:128.  Each gate matmul uses a K=128
block-diagonal weight blockdiag(Wg_h, Wg_h) computing both groups in one
pass; the 2-channel x contribution + gate bias (constant-1 channel)
accumulate via a K=6 block-diagonal matmul.

dtypes: the z and r gates matmul in fp8 e4m3 (PE streams fp8 at 2x the
bf16 column rate; their error is damped by the sigmoid slope 0.25 and
the small |h - hc| factor), the candidate/c gate matmuls stay bf16 (its
error passes to the output un-damped).  PSUM accumulation fp32,
activations + gating arithmetic bf16, output bf16 (upcast on host).

Perf structure (v3):
 - input DMAs on SP in need-order: aux2 (x data + x-weights, gates the
   first matmuls), aux1 (h^T block0 bf16 + full h^T fp8 + h-weights),
   htb (h^T block1 bf16, only needed by the later DVE work).  Per-engine
   DMA is byte-bound (~25 GB/s x 16 engines); descriptor-gen ~0.5-1.0us
   per DMA, serial per issuing engine.
 - PE warm-up burst keeps the HAM clock at 2.4 GHz while DMAs fly
 - gate order r, z, c; fp8 halves the z/r stream time
 - blend algebra: w = 1-z (fused tensor_scalar) and zh = z*h run BEFORE
   the tanh; only wc = w*hc and out = zh + wc remain on the post-tanh
   critical path (all on DVE; the Pool engine's tensor ops measured ~3x
   slower and contend with DVE for SBUF -- don't use it)
 - output: block 0 DMA on the SP HWDGE queue, block 1 on the ACT HWDGE
   queue, so the two ~0.6us descriptor generations overlap
"""

import ml_dtypes
import numpy as np

import concourse.tile as tile
from concourse import bacc, mybir
from concourse.bass_utils import run_bass_kernel_spmd

N_CORES = 8
B, N, IN_DIM, HID = 16, 1024, 2, 64
GC_ALPHA = 0.05
CIN = HID + IN_DIM          # 66
R = (B // N_CORES) * N      # 2048 rows per core
G = R // 2                  # 1024 rows per group (A/B)
BLK = 512                   # psum free-dim block
N_WARMUP_MM = 4

F32 = mybir.dt.float32
BF16 = mybir.dt.bfloat16
F8 = mybir.dt.float8e4
AF = mybir.ActivationFunctionType
ALU = mybir.AluOpType
BF16_NP = ml_dtypes.bfloat16
F8_NP = ml_dtypes.float8_e4m3fn

_program_cache = {}


def build_program():
    # Bacc (not raw Bass): its compile() runs move_matmul_waits_to_ldweights
    # + generate_event_semaphores, which split multi-sem waits to satisfy
    # the TRN2 "at most 1 sync wait per instruction" constraint.
    nc = bacc.Bacc()
    # aux2: x-side weights + x data: [wx_zr f8 | wx_c bf16 | xt bf16 | xt f8]
    aux2 = nc.dram_tensor("aux2", [6, 896], F32, kind="ExternalInput")
    # aux1: [h^T block0 bf16 | full h^T f8 | wb_zr f8 | wb_c bf16]
    aux1 = nc.dram_tensor("aux1", [128, 640], F32, kind="ExternalInput")
    # htb: h^T block1 bf16 (needed only by DVE rhb1/zh1/blend)
    htb = nc.dram_tensor("htb", [128, BLK], BF16, kind="ExternalInput")
    ot = nc.dram_tensor("ot", [128, G], BF16, kind="ExternalOutput")

    with tile.TileContext(nc) as tc:
        with (
            tc.tile_pool(name="sb", bufs=1) as sb,
            tc.tile_pool(name="ps", bufs=1, space="PSUM") as ps,
        ):
            AUX1 = sb.tile([128, 640], F32, tag="AUX1")
            AUX2 = sb.tile([6, 896], F32, tag="AUX2")
            HTB1 = sb.tile([128, BLK], BF16, tag="HTB1")
            RT = sb.tile([128, G], BF16, tag="RT")
            ZT = sb.tile([128, G], BF16, tag="ZT")
            WT = sb.tile([128, G], BF16, tag="WT")
            ZH = sb.tile([128, G], BF16, tag="ZH")
            RHB = sb.tile([128, G], BF16, tag="RHB")
            HC = sb.tile([128, G], BF16, tag="HC")
            WC = sb.tile([128, G], BF16, tag="WC")
            OT = sb.tile([128, G], BF16, tag="OT")
            WARM = sb.tile([128, BLK], BF16, tag="WARM")
            dummy = sb.tile([1, 1], F32, tag="dummy")

            HTB0 = AUX1[:, 0:256].bitcast(BF16)      # [128, 512]
            HF8 = AUX1[:, 256:512].bitcast(F8)       # [128, 1024]
            WBZR = AUX1[:, 512:576].bitcast(F8)      # [128, 256] z|r
            WBC = AUX1[:, 576:640].bitcast(BF16)     # [128, 128]
            WXZR = AUX2[:, 0:64].bitcast(F8)         # [6, 256] z|r
            WXC = AUX2[:, 64:128].bitcast(BF16)      # [6, 128]
            XTB = AUX2[:, 128:640].bitcast(BF16)     # [6, 1024]
            XTF = AUX2[:, 640:896].bitcast(F8)       # [6, 1024]

            # Input DMAs on SP in need-order: tiny aux2 first.
            nc.vector.memset(dummy, 0.0)
            nc.sync.dma_start(out=AUX2, in_=aux2[:, :])
            nc.sync.dma_start(out=AUX1, in_=aux1[:, :])
            nc.sync.dma_start(out=HTB1, in_=htb[:, :])

            # Fire the ACT table load (sigmoid_and_others, covers tanh)
            # immediately so it overlaps the input DMAs.
            nc.scalar.activation(
                out=dummy, in_=dummy, func=AF.Sigmoid, bias=dummy[0:1, 0:1]
            )

            # PE warm-up: dummy matmuls while DMAs are in flight keep the
            # HAM activity window busy so real matmuls run at 2.4 GHz.
            # They scribble into pc0's bank, which the c-gate matmul
            # overwrites (start=True) later.
            nc.vector.memset(WARM, 0.0)

            cols0 = slice(0, BLK)
            cols1 = slice(BLK, G)
            pr0 = ps.tile([128, BLK], F32, tag="pr0")
            pr1 = ps.tile([128, BLK], F32, tag="pr1")
            pz0 = ps.tile([128, BLK], F32, tag="pz0")
            pz1 = ps.tile([128, BLK], F32, tag="pz1")
            pc0 = ps.tile([128, BLK], F32, tag="pc0")
            pc1 = ps.tile([128, BLK], F32, tag="pc1")

            def mm8(psum_t, wt, data, cols, start):
                nc.tensor.matmul(
                    psum_t[:, 0:BLK], wt, data[:, cols],
                    start=start, stop=not start, skip_group_check=True,
                )

            for _ in range(N_WARMUP_MM):
                nc.tensor.matmul(
                    pc0[:, :], WARM[:, 0:128], WARM[:, :],
                    start=True, stop=True, skip_group_check=True,
                )

            # ---- flattened schedule ----
            # r gate (fp8): xb pair, then h pair as soon as aux1 lands
            mm8(pr0, WXZR[0:6, 128:256], XTF, cols0, True)
            mm8(pr1, WXZR[0:6, 128:256], XTF, cols1, True)
            mm8(pr0, WBZR[:, 128:256], HF8, cols0, False)
            mm8(pr1, WBZR[:, 128:256], HF8, cols1, False)
            nc.scalar.activation(out=RT[:, cols0], in_=pr0[:, :], func=AF.Sigmoid)
            nc.vector.tensor_mul(RHB[:, cols0], RT[:, cols0], HTB0[:, :])
            nc.scalar.activation(out=RT[:, cols1], in_=pr1[:, :], func=AF.Sigmoid)
            nc.vector.tensor_mul(RHB[:, cols1], RT[:, cols1], HTB1[:, :])

            # z gate (fp8)
            mm8(pz0, WXZR[0:6, 0:128], XTF, cols0, True)
            mm8(pz1, WXZR[0:6, 0:128], XTF, cols1, True)
            mm8(pz0, WBZR[:, 0:128], HF8, cols0, False)
            mm8(pz1, WBZR[:, 0:128], HF8, cols1, False)
            nc.scalar.activation(out=ZT[:, cols0], in_=pz0[:, :], func=AF.Sigmoid)
            nc.scalar.activation(out=ZT[:, cols1], in_=pz1[:, :], func=AF.Sigmoid)

            # c gate (bf16: its error is un-damped in the output)
            mm8(pc0, WXC[0:6, :], XTB, cols0, True)
            mm8(pc1, WXC[0:6, :], XTB, cols1, True)
            mm8(pc0, WBC[:, :], RHB, cols0, False)
            mm8(pc1, WBC[:, :], RHB, cols1, False)

            # pre-tanh blend work: w = 1-z (fused) and zh = z*h, so only
            # wc = w*hc and out = zh + wc remain after each tanh.
            nc.vector.tensor_scalar(
                out=WT[:, cols0], in0=ZT[:, cols0],
                scalar1=-1.0, scalar2=1.0, op0=ALU.mult, op1=ALU.add,
            )
            nc.vector.tensor_mul(ZH[:, cols0], ZT[:, cols0], HTB0[:, :])
            nc.vector.tensor_scalar(
                out=WT[:, cols1], in0=ZT[:, cols1],
                scalar1=-1.0, scalar2=1.0, op0=ALU.mult, op1=ALU.add,
            )
            nc.vector.tensor_mul(ZH[:, cols1], ZT[:, cols1], HTB1[:, :])

            nc.scalar.activation(out=HC[:, cols0], in_=pc0[:, :], func=AF.Tanh)
            nc.scalar.activation(out=HC[:, cols1], in_=pc1[:, :], func=AF.Tanh)

            nc.vector.tensor_mul(WC[:, cols0], WT[:, cols0], HC[:, cols0])
            nc.vector.tensor_add(OT[:, cols0], ZH[:, cols0], WC[:, cols0])
            nc.sync.dma_start(out=ot[:, cols0], in_=OT[:, cols0])

            nc.vector.tensor_mul(WC[:, cols1], WT[:, cols1], HC[:, cols1])
            nc.vector.tensor_add(OT[:, cols1], ZH[:, cols1], WC[:, cols1])
            # second output half on the ACT HWDGE queue: its descriptor-gen
            # overlaps SP's first-half descriptor-gen
            nc.scalar.dma_start(out=ot[:, cols1], in_=OT[:, cols1])

    nc.compile()
    return nc


def get_program():
    if "nc" not in _program_cache:
        _program_cache["nc"] = build_program()
    return _program_cache["nc"]


def fold_params(rnn_W, rnn_b):
    """Fold the gconv_rnn bug + gate sums into per-gate [66,64] weights."""
    Wf = rnn_W[:, :CIN, :] + GC_ALPHA * (
        rnn_W[:, CIN : 2 * CIN, :] + rnn_W[:, 2 * CIN : 3 * CIN, :]
    )  # [6, 66, 64]
    Wg = np.stack([Wf[0] + Wf[1], Wf[2] + Wf[3], Wf[4] + Wf[5]])  # [3,66,64]
    bg = np.stack(
        [rnn_b[0] + rnn_b[1], rnn_b[2] + rnn_b[3], rnn_b[4] + rnn_b[5]]
    )  # [3, 64]
    return Wg, bg


def make_in_maps(x, h, rnn_W, rnn_b):
    Wg, bg = fold_params(rnn_W, rnn_b)
    # combined = concat(x, h): channels 0:2 are x, 2:66 are h.
    # Gate order: z=0, r=1, c=2.
    W_x = Wg[:, :IN_DIM, :]  # [3, 2, 64]
    W_h = Wg[:, IN_DIM:, :]  # [3, 64, 64]

    # Block-diagonal weights: z|r packed fp8 (gate g in cols 128g:128g+128
    # of the 256-wide z|r block), c packed bf16.
    # wx rows per group: [x0; x1; 1] -> [Wg_x; bg] folds the bias in.
    wbzr = np.zeros((128, 256), F8_NP)
    wxzr = np.zeros((6, 256), F8_NP)
    wbc = np.zeros((128, 128), BF16_NP)
    wxc = np.zeros((6, 128), BF16_NP)
    for g in (0, 1):  # z, r
        wbzr[0:64, 128 * g : 128 * g + 64] = W_h[g]
        wbzr[64:128, 128 * g + 64 : 128 * g + 128] = W_h[g]
        wxzr[0:2, 128 * g : 128 * g + 64] = W_x[g]
        wxzr[2, 128 * g : 128 * g + 64] = bg[g]
        wxzr[3:5, 128 * g + 64 : 128 * g + 128] = W_x[g]
        wxzr[5, 128 * g + 64 : 128 * g + 128] = bg[g]
    wbc[0:64, 0:64] = W_h[2]
    wbc[64:128, 64:128] = W_h[2]
    wxc[0:2, 0:64] = W_x[2]
    wxc[2, 0:64] = bg[2]
    wxc[3:5, 64:128] = W_x[2]
    wxc[5, 64:128] = bg[2]

    hf = h.reshape(N_CORES, R, HID)
    xf = x.reshape(N_CORES, R, IN_DIM)
    in_maps = []
    for c in range(N_CORES):
        ht_host = np.ascontiguousarray(
            np.concatenate([hf[c, :G].T, hf[c, G:].T], axis=0)
        ).astype(BF16_NP)  # [128, G] bf16
        aux1_host = np.empty((128, 640), np.float32)
        aux1_host[:, 0:256] = ht_host[:, 0:BLK].view(np.float32)
        aux1_host[:, 256:512] = (
            ht_host.astype(F8_NP).view(np.float32)
        )
        aux1_host[:, 512:576] = wbzr.view(np.float32)
        aux1_host[:, 576:640] = wbc.view(np.float32)
        xt_host = np.empty((6, G), BF16_NP)
        xt_host[0:2] = xf[c, :G].T
        xt_host[2] = 1.0
        xt_host[3:5] = xf[c, G:].T
        xt_host[5] = 1.0
        aux2_host = np.empty((6, 896), np.float32)
        aux2_host[:, 0:64] = wxzr.view(np.float32)
        aux2_host[:, 64:128] = wxc.view(np.float32)
        aux2_host[:, 128:640] = xt_host.view(np.float32)
        aux2_host[:, 640:896] = xt_host.astype(F8_NP).view(np.float32)
        in_maps.append(
            dict(
                aux2=aux2_host,
                aux1=aux1_host,
                htb=np.ascontiguousarray(ht_host[:, BLK:G]),
            )
        )
    return in_maps


def gather_output(results):
    outs = []
    for c in range(N_CORES):
        o = np.asarray(results[c]["ot"]).astype(np.float32)  # [128, G]
        outs.append(np.concatenate([o[:64].T, o[64:].T], axis=0))  # [R, HID]
    return (
        np.concatenate(outs, axis=0).reshape(B, N, HID).astype(np.float32)
    )


def run(inputs, trace=False, **kw):
    x = np.ascontiguousarray(np.asarray(inputs["x"], dtype=np.float32))
    h = np.ascontiguousarray(
        np.asarray(inputs["hidden_state"], dtype=np.float32)
    )
    rnn_W = np.asarray(inputs["rnn_W"], dtype=np.float32)
    rnn_b = np.asarray(inputs["rnn_b"], dtype=np.float32)

    in_maps = make_in_maps(x, h, rnn_W, rnn_b)
    nc = get_program()
    res = run_bass_kernel_spmd(
        nc, in_maps, core_ids=list(range(N_CORES)), trace=trace, **kw
    )
    return gather_output(res.results), res


def kernel(**inputs) -> np.ndarray:
    out, _ = run(inputs)
    return out
